# revision 14
# baseline (speedup 1.0000x reference)
"""Trainium2 Bass kernel for nn_CodedNet.

Reference computation (B=256, P=64, C=31):
    roll x per-channel along i, multiply by tiled sign mask, roll back,
    sum over channels.
The rolls cancel on x and only shift the mask, so the whole net collapses
to:
    out[b,i,j] = sum_c x[b,i,j,c] * mask[(i-c) % P, j]
with mask = tile(sign(w).reshape(32,32), (2,2))  (values in {-1,+1}).

Strategy: pure data parallel over batch (32 batches per core, 8 cores).
Per core x is viewed as [2048 rows=(b,i), 1984 cols=(j,c)] and processed
in 16 [128, 1984] SBUF tiles, all resident:

    sync+scalar : 16 HWDGE loads split across BOTH HW-DGE rings
                  (measured 348 GB/s/core vs 284 on one ring)
    vector      : per quad of 4 tiles, one fat in-place sign multiply
                  (wt broadcast via step-0 AP) and one fat segmented
                  reduce over c ([128, 4, 64, 31] -> [128, 4, 64]);
                  8 DVE ops per pass instead of 32 (per-op overhead was
                  ~30% of DVE time)
    gpsimd      : one SWDGE store of the [128, 16*64] output buffer

The sign tile WT[p, j*31+c] = mask[(p%64 - c)%64, j] is identical for
every row-tile (row%64 == p%64 when tiles are 128 rows), loaded once.
Consecutive DVE ops need no semaphores: the engine's per-op pipeline
DRAIN orders same-engine RAW (verified bit-stable on HW).

Raw bass (no TileContext): the walrus codegen used by the axon/PJRT path
allows only one sync wait per instruction, so all waits are standalone
wait_ge ops and DMAs get dedicated semaphores. Semaphores are cleared in
a preamble (they persist across NEFF executions) behind an NRT
pseudo-barrier.

`iters > 1` repeats the pipeline with cumulative semaphore thresholds
(x reloaded from DRAM each iteration, output buffer halves alternated) —
used by bench.py to measure steady-state per-iteration HW time, since
the axon path has no NTFF profiling.
"""

import sys

sys.path.insert(0, "/opt/trn_rl_repo")

import numpy as np

B, P, C = 256, 64, 31
N_CORES = 8
ROWS_PER_CORE = (B // N_CORES) * P          # 2048
FREE = P * C                                 # 1984
N_TILES = ROWS_PER_CORE // 128               # 16
N_QUADS = N_TILES // 4                       # 4

_CACHE = {}


def _build_program(iters: int = 1):
    """Build the Bass program (shared by all cores, SPMD)."""
    import concourse.bass as bass
    import concourse.mybir as mybir
    from contextlib import ExitStack

    nc = bass.Bass()
    x_h = nc.declare_dram_parameter("x", [ROWS_PER_CORE, FREE], mybir.dt.float32, isOutput=False)
    wt_h = nc.declare_dram_parameter("wt", [128, FREE], mybir.dt.float32, isOutput=False)
    out_h = nc.declare_dram_parameter("out", [ROWS_PER_CORE, P], mybir.dt.float32, isOutput=True)

    x_t = x_h[:, :].rearrange("(n p) f -> n p f", p=128)
    # out[128t + p, j] viewed as [p, t, j] so one SBUF buffer stores all tiles
    out_t = out_h[:, :].rearrange("(n p) f -> p n f", p=128)

    ctx = ExitStack()
    with ctx:
        x_sb = ctx.enter_context(nc.sbuf_tensor([128, N_TILES * FREE], mybir.dt.float32))
        wt_sb = ctx.enter_context(nc.sbuf_tensor([128, FREE], mybir.dt.float32))
        o_sb = ctx.enter_context(nc.sbuf_tensor([128, 2 * N_TILES * P], mybir.dt.float32))

        sem_w = ctx.enter_context(nc.semaphore("sem_w"))
        sem_x = [ctx.enter_context(nc.semaphore(f"sem_x{t}")) for t in range(N_TILES)]
        sem_g = [ctx.enter_context(nc.semaphore(f"sem_g{q}")) for q in range(N_QUADS)]
        sem_out = ctx.enter_context(nc.semaphore("sem_out"))

        # Clear all semaphores at program start (values persist across NEFF
        # executions), then an NRT pseudo-barrier so no engine can pass a
        # wait on a stale value before the clears land.
        for s in [sem_w, sem_out, *sem_x, *sem_g]:
            nc.sync.sem_clear(s)
        nc._nrt_pseudo_barrier()

        block = ctx.enter_context(nc.Block())

        def xs(t):
            return x_sb[:, t * FREE:(t + 1) * FREE]

        @block.sync
        def _(sync):
            sync.dma_start(out=wt_sb[:, :], in_=wt_h[:, :]).then_inc(sem_w, 16)
            for k in range(iters):
                for t in range(0, N_TILES, 2):
                    if k >= 1:
                        # slot reused across iterations: previous reduce done
                        sync.wait_ge(sem_g[t // 4], k)
                    sync.dma_start(out=xs(t), in_=x_t[t]).then_inc(sem_x[t], 16)

        @block.scalar
        def _(scalar):
            for k in range(iters):
                for t in range(1, N_TILES, 2):
                    if k >= 1:
                        scalar.wait_ge(sem_g[t // 4], k)
                    scalar.dma_start(out=xs(t), in_=x_t[t]).then_inc(sem_x[t], 16)

        @block.vector
        def _(vector):
            vector.wait_ge(sem_w, 16)
            for k in range(iters):
                for q in range(N_QUADS):
                    if q == 0 and k >= 2:
                        # o_sb half reuse: store of iteration k-2 done
                        vector.wait_ge(sem_out, 16 * (k - 1))
                    vector.wait_ge(sem_x[4 * q + 2], 16 * (k + 1))
                    vector.wait_ge(sem_x[4 * q + 3], 16 * (k + 1))
                    quad = x_sb[:, 4 * q * FREE:(4 * q + 4) * FREE]
                    wt_b = wt_sb[:, :].unsqueeze(1).broadcast_to([128, 4, FREE])
                    nc.vector.tensor_mul(
                        quad.rearrange("p (n f) -> p n f", n=4),
                        quad.rearrange("p (n f) -> p n f", n=4),
                        wt_b,
                    )
                    off = (k % 2) * N_TILES * P
                    nc.vector.reduce_sum(
                        o_sb[:, off + 4 * q * P:off + (4 * q + 4) * P]
                            .rearrange("p (n f) -> p n f", n=4),
                        quad.rearrange("p (n j c) -> p n j c", n=4, c=C),
                        axis=mybir.AxisListType.X,
                    ).then_inc(sem_g[q], 1)

        @block.gpsimd
        def _(gpsimd):
            for k in range(iters):
                # DVE reduces are in-order: last quad's sem covers all
                gpsimd.wait_ge(sem_g[N_QUADS - 1], k + 1)
                if k >= 1:
                    gpsimd.wait_ge(sem_out, 16 * k)
                gpsimd.dma_start(
                    out=out_t,
                    in_=o_sb[:, (k % 2) * N_TILES * P:((k % 2) + 1) * N_TILES * P]
                        .rearrange("p (n f) -> p n f", f=P),
                ).then_inc(sem_out, 16)
            gpsimd.wait_ge(sem_out, 16 * iters)
    return nc


def _get_program(iters: int = 1):
    key = ("nc", iters)
    if key not in _CACHE:
        _CACHE[key] = _build_program(iters)
    return _CACHE[key]


def _sign_tile(w: np.ndarray) -> np.ndarray:
    mask = np.tile(np.sign(w.astype(np.float32)).reshape(32, 32), (2, 2))  # [64, 64] = (r, j)
    i_idx = np.arange(128) % P
    c_idx = np.arange(C)
    j_idx = np.arange(P)
    wt = mask[(i_idx[:, None, None] - c_idx[None, None, :]) % P, j_idx[None, :, None]]
    return np.ascontiguousarray(wt.reshape(128, FREE).astype(np.float32))


def kernel(x: np.ndarray, w: np.ndarray) -> np.ndarray:
    from concourse.bass_utils import run_bass_kernel_spmd

    nc = _get_program()
    wt = _sign_tile(w)
    x2 = np.ascontiguousarray(x.astype(np.float32, copy=False)).reshape(B * P, FREE)
    in_maps = [
        {"x": x2[k * ROWS_PER_CORE:(k + 1) * ROWS_PER_CORE], "wt": wt}
        for k in range(N_CORES)
    ]
    res = run_bass_kernel_spmd(nc, in_maps, list(range(N_CORES)))
    out = np.concatenate([res.results[k]["out"] for k in range(N_CORES)], axis=0)
    return out.reshape(B, P, P)
